# revision 21
# baseline (speedup 1.0000x reference)
"""ANFIS kernel for 8 TRN2 NeuronCores — pure batch data-parallel.

Math: out[b,o] = prod_f(x[b,f]) * w[b,o]^32 where
  w = sum_r(p_r * m_r) / sum_r(m_r),  m_r = exp(-((y-c_r)/s_r)^2),
  y = MLP(x).  exp(-z^2) is computed on the ScalarEngine as
  Derivative_Erf(scale*y + bias) (= 2/sqrt(pi) * exp(-z^2); the constant
  cancels in the normalization).  D = sum m and N = sum p*m are reduced
  over rules with fp16 TensorE matmuls (identity / diag(p) stationary),
  accumulating in f32 PSUM.

v4 schedule (perfetto-driven):
  - DERF reading y from SBUF is ~120ns/op faster than from PSUM, so both
    o-tiles' y are copied PSUM->SBUF on the idle DVE; only rule 0 of
    o-tile 0 reads PSUM (the copy is still in flight then).
  - PE p-state resets on any >100ns idle gap, so zero-weight filler
    matmuls (0-stationary x anything accumulates nothing) bridge the
    relu waits at the L2 and L3 boundaries, keeping the MLP at full clock.
  - PSUM bank choreography: 8 banks = ps_a(2) + ps_b(2) + ps_mlp(4) during
    the MLP, and ps_a/ps_b tag rotation hands yT0->N1 and scratch->yT1p->D1
    so the second o-tile's D/N accumulation never stalls.
  - tail: recips + w-mults + one squaring ladder + output mults on DVE,
    the other ladder on ACT right after its last DERF; GpSimd is avoided
    late-kernel (library reload + dispatch latency measured ~6us).
"""
import sys

if "/opt/trn_rl_repo" not in sys.path:
    sys.path.insert(0, "/opt/trn_rl_repo")

import numpy as np
import ml_dtypes
ml_bf16 = ml_dtypes.bfloat16

import concourse.bacc as bacc
import concourse.mybir as mybir
from concourse.bass_utils import run_bass_kernel_spmd
from concourse.tile import TileContext
from concourse.mybir import AluOpType as Op

B, IN_DIM, OUT_DIM, N_RULES, H = 8192, 32, 256, 16, 256
N_CORES = 8
BL = B // N_CORES          # 1024 batch rows per core
P = 128                    # partitions
NOT = OUT_DIM // P         # 2 o-tiles
NJ = H // P                # 2 hidden j-tiles
FD = 512                   # matmul free-dim chunk (one PSUM bank)
F32 = mybir.dt.float32
F16 = mybir.dt.float16
BF16 = mybir.dt.bfloat16

# packed f32 constant columns: b1t | b2t | scl | bia
C_B1 = 0
C_B2 = C_B1 + NJ
C_SCL = C_B2 + NJ
C_BIA = C_SCL + NOT * N_RULES
C_END = C_BIA + NOT * N_RULES

_nc_cache = None


def _build():
    global _nc_cache
    if _nc_cache is not None:
        return _nc_cache
    nc = bacc.Bacc(None, target_bir_lowering=False, debug=False, num_devices=N_CORES)

    xw_d = nc.declare_dram_parameter("xw", [3 * IN_DIM, BL], BF16, isOutput=False)
    w1s_d = nc.declare_dram_parameter("w1s", [3 * IN_DIM, H], BF16, isOutput=False)
    cst_d = nc.declare_dram_parameter("cst", [P, C_END], F32, isOutput=False)
    w23_d = nc.declare_dram_parameter("w23", [P, (NJ * NJ + NJ * NOT) * P], F16, isOutput=False)
    f16c_d = nc.declare_dram_parameter("f16c", [P, P + NOT * N_RULES * P], F16, isOutput=False)
    xbp_d = nc.declare_dram_parameter("xbp", [P, (BL // P) * IN_DIM], F32, isOutput=False)
    out_d = nc.declare_dram_parameter("out", [OUT_DIM, BL], F32, isOutput=True)

    DERF = mybir.ActivationFunctionType.Derivative_Erf
    SQ = mybir.ActivationFunctionType.Square
    RELU = mybir.ActivationFunctionType.Relu
    NCH = BL // FD  # chunks

    with TileContext(nc) as tc:
        with tc.sbuf_pool(name="sb", bufs=1) as sb:
            # ---- warmup first in program order: table preload + PE ramp.
            # junkbf is bf16 zeros: warm matmuls and MLP fillers both use it.
            junkbf = sb.tile([P, P + 256], BF16)
            nc.vector.memset(junkbf[:], 0.0)
            nc.scalar.activation(junkbf[:, 0:8], junkbf[:, 0:8], DERF)
            # pin the m-pool's SBUF region now, while hT/h2T don't exist yet —
            # otherwise the allocator overlaps them and the first m tiles
            # inherit a wait on h2T's last reader (measured 1.9us stall)
            mwarm = sb.tile([P, BL], F16, tag="m", bufs=8, name="mwarm")
            nc.vector.memset(mwarm[:, 0:8], 0.0)
            with tc.psum_pool(name="ps_warm", bufs=1) as ps_warm:
                wt = ps_warm.tile([P, 256], F32, tag="warm")
                for _ in range(6):
                    nc.tensor.matmul(wt[:], junkbf[:, :P], junkbf[:, P:], start=True, stop=True)

            # ---- input DMAs (arrival is gated by DMA-ring startup, not by
            # the issuing sequencer; keep them all on the idle SP queue).
            # xw is split in two in case ring assignment is per-DMA. ----
            xw = sb.tile([3 * IN_DIM, BL], BF16)
            nc.sync.dma_start(out=xw[0:48, :], in_=xw_d[0:48, :])
            nc.sync.dma_start(out=xw[48:96, :], in_=xw_d[48:96, :])
            w1s = sb.tile([3 * IN_DIM, H], BF16)
            nc.sync.dma_start(out=w1s[:], in_=w1s_d[:])
            cst = sb.tile([P, C_END], F32)
            nc.sync.dma_start(out=cst[:], in_=cst_d[:])
            w23 = sb.tile([P, (NJ * NJ + NJ * NOT) * P], F16)
            nc.sync.dma_start(out=w23[:], in_=w23_d[:])
            xbp = sb.tile([P, (BL // P) * IN_DIM], F32)
            nc.sync.dma_start(out=xbp[:], in_=xbp_d[:])
            f16c = sb.tile([P, P + NOT * N_RULES * P], F16)
            nc.sync.dma_start(out=f16c[:], in_=f16c_d[:])

            b1t = cst[:, C_B1:C_B1 + NJ]
            b2t = cst[:, C_B2:C_B2 + NJ]
            scl = cst[:, C_SCL:C_SCL + NOT * N_RULES]
            bia = cst[:, C_BIA:C_BIA + NOT * N_RULES]
            eye16 = f16c[:, :P]
            dgs = f16c[:, P:]

            def W2blk(k, j):
                return w23[:, (k * NJ + j) * P:(k * NJ + j + 1) * P]

            def W3blk(k, j):
                off = NJ * NJ * P
                return w23[:, off + (k * NOT + j) * P:off + (k * NOT + j + 1) * P]

            def relu_bias_chunk(dst, src_psum, bias_col, c, eng):
                cs = slice(c * FD, (c + 1) * FD)
                if eng == 0:
                    nc.vector.tensor_scalar(dst[:, cs], src_psum[:, cs], bias_col, 0.0,
                                            Op.add, Op.max)
                else:
                    nc.scalar.activation(dst[:, cs], src_psum[:, cs], RELU,
                                         bias=bias_col, scale=1.0)

            hT = [None, None]
            h2T = [None, None]
            # ps_a: l2_j0 -> yT0 -> N1 ; ps_b: scratch(fillers) -> yT1p -> D1
            with tc.psum_pool(name="ps_a", bufs=1) as ps_a:
                with tc.psum_pool(name="ps_b", bufs=1) as ps_b:
                    # filler scratch shares ps_b's banks via tag rotation;
                    # nothing reads it so successors allocate instantly
                    wt2 = ps_b.tile([P, FD], F32, tag="bb", name="wt2")

                    def filler(n):
                        for _ in range(n):
                            nc.tensor.matmul(wt2[:, :256], junkbf[:, :P], junkbf[:, P:],
                                             start=True, stop=True)

                    with tc.psum_pool(name="ps_mlp", bufs=2) as ps_mlp:
                        l1s = [None, None]
                        for j in range(NJ):
                            l1 = ps_mlp.tile([P, BL], F32, tag="mlp", name=f"l1_{j}")
                            for c in range(NCH):
                                nc.tensor.matmul(
                                    l1[:, c * FD:(c + 1) * FD],
                                    w1s[:, j * P:(j + 1) * P],
                                    xw[:, c * FD:(c + 1) * FD],
                                    start=True, stop=True,
                                )
                            l1s[j] = l1
                            hT[j] = sb.tile([P, BL], F16, name=f"hT{j}")
                        # crosswise engine split: each engine gets one chunk
                        # of each j, so neither serializes a whole j-tile
                        relu_bias_chunk(hT[1], l1s[1], b1t[:, 1:2], 0, 1)  # ACT
                        relu_bias_chunk(hT[1], l1s[1], b1t[:, 1:2], 1, 0)  # DVE
                        relu_bias_chunk(hT[0], l1s[0], b1t[:, 0:1], 0, 0)  # DVE
                        relu_bias_chunk(hT[0], l1s[0], b1t[:, 0:1], 1, 1)  # ACT
                        filler(10)  # bridge the relu wait; keep PE clocked up
                        l2s = [None, None]
                        for j in (1, 0):  # j1 first: its relus run on ACT
                            l2 = ps_mlp.tile([P, BL], F32, tag="mlp", name=f"l2_{j}")
                            for c in range(NCH):
                                for k in range(NJ):
                                    nc.tensor.matmul(
                                        l2[:, c * FD:(c + 1) * FD],
                                        W2blk(k, j),
                                        hT[k][:, c * FD:(c + 1) * FD],
                                        start=(k == 0), stop=(k == NJ - 1),
                                    )
                            l2s[j] = l2
                            h2T[j] = sb.tile([P, BL], F16, name=f"h2T{j}")
                        relu_bias_chunk(h2T[1], l2s[1], b2t[:, 1:2], 0, 1)  # ACT
                        relu_bias_chunk(h2T[1], l2s[1], b2t[:, 1:2], 1, 0)  # DVE
                        relu_bias_chunk(h2T[0], l2s[0], b2t[:, 0:1], 0, 0)  # DVE
                        relu_bias_chunk(h2T[0], l2s[0], b2t[:, 0:1], 1, 1)  # ACT

                        # P[b] = prod_f x[b,f] -> replicated [P, BL]
                        P_rep = sb.tile([P, BL], F32)
                        P_all = sb.tile([P, BL // P], F32)
                        nc.vector.tensor_reduce(
                            P_all[:],
                            xbp.rearrange("p (t f) -> p t f", f=IN_DIM),
                            mybir.AxisListType.X, Op.mult,
                        )
                        P_row = sb.tile([1, BL], F32)
                        for t in range(BL // P):
                            nc.sync.dma_start(out=P_row[0:1, t * P:(t + 1) * P], in_=P_all[:, t:t + 1])
                        nc.gpsimd.partition_broadcast(P_rep[:], P_row[0:1, :])

                    # bridge the relu2 wait before L3.  rhs = h2T[1] makes
                    # these depend on L2-j1's relu so the scheduler cannot
                    # hoist them back before L2 (standalone fillers get
                    # reordered to the earliest idle slot).
                    for _ in range(6):
                        nc.tensor.matmul(wt2[:, :256], junkbf[:, :P], h2T[1][:, :256],
                                         start=True, stop=True)
                    # ---- L3: yT0 (ps_a), yT1 (ps_b) -> both copied to SBUF
                    yT0 = ps_a.tile([P, BL], F32, tag="aa", name="yT0")
                    for c in range(NCH):
                        for k in range(NJ):
                            nc.tensor.matmul(
                                yT0[:, c * FD:(c + 1) * FD], W3blk(k, 0),
                                h2T[k][:, c * FD:(c + 1) * FD],
                                start=(k == 0), stop=(k == NJ - 1),
                            )
                    yT1p = ps_b.tile([P, BL], F32, tag="bb", name="yT1p")
                    for c in range(NCH):
                        for k in range(NJ):
                            nc.tensor.matmul(
                                yT1p[:, c * FD:(c + 1) * FD], W3blk(k, 1),
                                h2T[k][:, c * FD:(c + 1) * FD],
                                start=(k == 0), stop=(k == NJ - 1),
                            )
                    # y PSUM->SBUF copies are emitted inside the rule loop,
                    # after rule 0's DERF, so the scheduler cannot gate the
                    # first DERF on them
                    y0s = sb.tile([P, BL], F32, name="y0s")
                    y1s = sb.tile([P, BL], F32, name="y1s")

                    # ---- memberships + D/N + w per o-tile ----
                    with tc.psum_pool(name="ps_c", bufs=1) as ps_c:
                        for ot in range(NOT):
                            if ot == 0:
                                D = ps_c.tile([P, BL], F32, tag="D", name="D0")
                                N = ps_c.tile([P, BL], F32, tag="N", name="N0")
                            else:
                                D = ps_b.tile([P, BL], F32, tag="bb", name="D1")
                                N = ps_a.tile([P, BL], F32, tag="aa", name="N1")
                            for r in range(N_RULES):
                                idx = ot * N_RULES + r
                                # rules 0-2 of ot0 read PSUM y (the SBUF
                                # copies are still in flight); rest read SBUF
                                ysrc = yT0 if (ot == 0 and r < 3) else (y0s if ot == 0 else y1s)
                                m = sb.tile([P, BL], F16, tag="m", bufs=8, name=f"m{idx}")
                                if ot == NOT - 1 and r == N_RULES - 1:
                                    for c in range(NCH):
                                        cs = slice(c * FD, (c + 1) * FD)
                                        nc.scalar.activation(
                                            m[:, cs], ysrc[:, cs], DERF,
                                            bias=bia[:, idx:idx + 1], scale=scl[:, idx:idx + 1],
                                        )
                                else:
                                    nc.scalar.activation(
                                        m[:], ysrc[:], DERF,
                                        bias=bia[:, idx:idx + 1], scale=scl[:, idx:idx + 1],
                                    )
                                for c in range(NCH):
                                    cs = slice(c * FD, (c + 1) * FD)
                                    nc.tensor.matmul(D[:, cs], eye16, m[:, cs],
                                                     start=(r == 0), stop=(r == N_RULES - 1))
                                    nc.tensor.matmul(N[:, cs], dgs[:, idx * P:(idx + 1) * P], m[:, cs],
                                                     start=(r == 0), stop=(r == N_RULES - 1))
                                if ot == 0 and r == 0:
                                    nc.vector.tensor_copy(y0s[:, 0:FD], yT0[:, 0:FD])
                                    nc.vector.tensor_copy(y0s[:, FD:], yT0[:, FD:])
                                    nc.vector.tensor_copy(y1s[:], yT1p[:])
                            rD = sb.tile([P, BL], F32, tag="rD", bufs=2, name=f"rD{ot}")
                            w = sb.tile([P, BL], F32, tag="w", bufs=2, name=f"w{ot}")
                            o = sb.tile([P, BL], F32, tag="osb", bufs=2, name=f"osb{ot}")
                            if ot < NOT - 1:
                                # fully hidden under the next o-tile's DERFs
                                nc.vector.reciprocal_approx_fast(rD[:], D[:])
                                nc.vector.tensor_tensor(w[:], N[:], rD[:], Op.mult)
                                for _ in range(5):
                                    nc.vector.tensor_tensor(w[:], w[:], w[:], Op.mult)
                                nc.vector.tensor_tensor(o[:], w[:], P_rep[:], Op.mult)
                                nc.sync.dma_start(out=out_d[ot * P:(ot + 1) * P, :], in_=o[:])
                            else:
                                c0 = slice(0, FD)
                                c1 = slice(FD, BL)
                                # DVE: recips, w-mults, ladder c1, o-mults;
                                # ACT: ladder c0 right after its final DERF
                                # both recips first (c1's chain is critical),
                                # then both w-mults; ladder c0 on ACT (free
                                # right after its last DERF), c1 on DVE;
                                # o1 first so its DMA starts earliest
                                nc.vector.reciprocal_approx_fast(rD[:, c0], D[:, c0])
                                nc.vector.reciprocal_approx_fast(rD[:, c1], D[:, c1])
                                nc.vector.tensor_tensor(w[:, c0], N[:, c0], rD[:, c0], Op.mult)
                                nc.vector.tensor_tensor(w[:, c1], N[:, c1], rD[:, c1], Op.mult)
                                for _ in range(5):
                                    nc.scalar.activation(w[:, c0], w[:, c0], SQ)
                                for _ in range(4):
                                    nc.vector.tensor_tensor(w[:, c1], w[:, c1], w[:, c1], Op.mult)
                                nc.scalar.activation(w[:, c1], w[:, c1], SQ)
                                nc.vector.tensor_tensor(o[:, c0], w[:, c0], P_rep[:, c0], Op.mult)
                                nc.sync.dma_start(out=out_d[ot * P:(ot + 1) * P, c0], in_=o[:, c0])
                                nc.vector.tensor_tensor(o[:, c1], w[:, c1], P_rep[:, c1], Op.mult)
                                nc.sync.dma_start(out=out_d[ot * P:(ot + 1) * P, c1], in_=o[:, c1])

    nc.finalize()
    _nc_cache = nc
    return nc


def _prepare_in_maps(x, W1, b1, W2, b2, W3, b3, centers, widths, params):
    x = np.ascontiguousarray(x, dtype=np.float32)
    W1 = np.asarray(W1, np.float32); b1 = np.asarray(b1, np.float32)
    W2 = np.asarray(W2, np.float32); b2 = np.asarray(b2, np.float32)
    W3 = np.asarray(W3, np.float32); b3 = np.asarray(b3, np.float32)
    centers = np.asarray(centers, np.float32)
    widths = np.asarray(widths, np.float32)
    params = np.asarray(params, np.float32)

    def pack_w(W, nj_out):
        blocks = []
        for k in range(W.shape[0] // P):
            for j in range(nj_out):
                blocks.append(W[k * P:(k + 1) * P, j * P:(j + 1) * P])
        return np.concatenate(blocks, axis=1)

    w23 = np.ascontiguousarray(
        np.concatenate([pack_w(W2, NJ), pack_w(W3, NOT)], axis=1).astype(np.float16))

    b1t = b1.reshape(NJ, P).T
    b2t = b2.reshape(NJ, P).T
    inv = (1.0 / widths).astype(np.float32)
    biasf = ((b3[:, None] - centers) * inv).astype(np.float32)
    scl = inv.reshape(NOT, P, N_RULES).transpose(1, 0, 2).reshape(P, NOT * N_RULES)
    bia = biasf.reshape(NOT, P, N_RULES).transpose(1, 0, 2).reshape(P, NOT * N_RULES)
    cst = np.ascontiguousarray(np.concatenate([b1t, b2t, scl, bia], axis=1))

    ph = params.astype(np.float16)
    dgs = np.zeros((P, NOT * N_RULES * P), np.float16)
    for ot in range(NOT):
        for r in range(N_RULES):
            idx = ot * N_RULES + r
            dgs[:, idx * P:(idx + 1) * P] = np.diag(ph[ot * P:(ot + 1) * P, r])
    f16c = np.ascontiguousarray(np.concatenate([np.eye(P, dtype=np.float16), dgs], axis=1))

    # L1 bf16 hi/lo stacking: y1 = W1h.T@xh + W1l.T@xh + W1h.T@xl
    W1h = W1.astype(ml_bf16)
    W1l = (W1 - W1h.astype(np.float32)).astype(ml_bf16)
    w1s = np.ascontiguousarray(np.concatenate([W1h, W1l, W1h], axis=0))  # [96, H]

    in_maps = []
    for i in range(N_CORES):
        xs = x[i * BL:(i + 1) * BL]                              # [BL, 32]
        xT = np.ascontiguousarray(xs.T)                          # [32, BL]
        xh = xT.astype(ml_bf16)
        xl = (xT - xh.astype(np.float32)).astype(ml_bf16)
        xw = np.ascontiguousarray(np.concatenate([xh, xh, xl], axis=0))  # [96, BL]
        xbp = np.ascontiguousarray(
            xs.reshape(BL // P, P, IN_DIM).transpose(1, 0, 2).reshape(P, -1))
        in_maps.append(dict(xw=xw, w1s=w1s, cst=cst, w23=w23, f16c=f16c, xbp=xbp))
    return in_maps


def run(trace=False, **inputs):
    nc = _build()
    in_maps = _prepare_in_maps(**inputs)
    res = run_bass_kernel_spmd(nc, in_maps, core_ids=list(range(N_CORES)), trace=trace)
    outs = [res.results[i]["out"].T for i in range(N_CORES)]     # each [BL, O]
    full = np.ascontiguousarray(np.concatenate(outs, axis=0), dtype=np.float32)
    return full, res


def kernel(**inputs) -> np.ndarray:
    full, _ = run(trace=False, **inputs)
    return full


# revision 24
# speedup vs baseline: 1.0280x; 1.0280x over previous
"""ANFIS kernel for 8 TRN2 NeuronCores — pure batch data-parallel.

Math: out[b,o] = prod_f(x[b,f]) * w[b,o]^32 where
  w = sum_r(p_r * m_r) / sum_r(m_r),  m_r = exp(-((y-c_r)/s_r)^2),
  y = MLP(x).  exp(-z^2) is computed on the ScalarEngine as
  Derivative_Erf(scale*y + bias) (= 2/sqrt(pi) * exp(-z^2); the constant
  cancels in the normalization).  D = sum m and N = sum p*m are reduced
  over rules with fp16 TensorE matmuls (identity / diag(p) stationary),
  accumulating in f32 PSUM.

v4 schedule (perfetto-driven):
  - DERF reading y from SBUF is ~120ns/op faster than from PSUM, so both
    o-tiles' y are copied PSUM->SBUF on the idle DVE; only rule 0 of
    o-tile 0 reads PSUM (the copy is still in flight then).
  - PE p-state resets on any >100ns idle gap, so zero-weight filler
    matmuls (0-stationary x anything accumulates nothing) bridge the
    relu waits at the L2 and L3 boundaries, keeping the MLP at full clock.
  - PSUM bank choreography: 8 banks = ps_a(2) + ps_b(2) + ps_mlp(4) during
    the MLP, and ps_a/ps_b tag rotation hands yT0->N1 and scratch->yT1p->D1
    so the second o-tile's D/N accumulation never stalls.
  - tail: recips + w-mults + one squaring ladder + output mults on DVE,
    the other ladder on ACT right after its last DERF; GpSimd is avoided
    late-kernel (library reload + dispatch latency measured ~6us).
"""
import sys

if "/opt/trn_rl_repo" not in sys.path:
    sys.path.insert(0, "/opt/trn_rl_repo")

import numpy as np
import ml_dtypes
ml_bf16 = ml_dtypes.bfloat16

import concourse.bacc as bacc
import concourse.mybir as mybir
from concourse.bass_utils import run_bass_kernel_spmd
from concourse.tile import TileContext
from concourse.mybir import AluOpType as Op

B, IN_DIM, OUT_DIM, N_RULES, H = 8192, 32, 256, 16, 256
N_CORES = 8
BL = B // N_CORES          # 1024 batch rows per core
P = 128                    # partitions
NOT = OUT_DIM // P         # 2 o-tiles
NJ = H // P                # 2 hidden j-tiles
FD = 512                   # matmul free-dim chunk (one PSUM bank)
F32 = mybir.dt.float32
F16 = mybir.dt.float16
BF16 = mybir.dt.bfloat16

# packed f32 constant columns: b1t | b2t | scl | bia
C_B1 = 0
C_B2 = C_B1 + NJ
C_SCL = C_B2 + NJ
C_BIA = C_SCL + NOT * N_RULES
C_END = C_BIA + NOT * N_RULES

_nc_cache = None


def _build():
    global _nc_cache
    if _nc_cache is not None:
        return _nc_cache
    nc = bacc.Bacc(None, target_bir_lowering=False, debug=False, num_devices=N_CORES)

    xw_d = nc.declare_dram_parameter("xw", [3 * IN_DIM, BL], BF16, isOutput=False)
    w1s_d = nc.declare_dram_parameter("w1s", [3 * IN_DIM, H], BF16, isOutput=False)
    cst_d = nc.declare_dram_parameter("cst", [P, C_END], F32, isOutput=False)
    w23_d = nc.declare_dram_parameter("w23", [P, (NJ * NJ + NJ * NOT) * P], F16, isOutput=False)
    f16c_d = nc.declare_dram_parameter("f16c", [P, P + NOT * N_RULES * P], F16, isOutput=False)
    xbp_d = nc.declare_dram_parameter("xbp", [P, (BL // P) * IN_DIM], F32, isOutput=False)
    out_d = nc.declare_dram_parameter("out", [OUT_DIM, BL], F32, isOutput=True)

    DERF = mybir.ActivationFunctionType.Derivative_Erf
    SQ = mybir.ActivationFunctionType.Square
    RELU = mybir.ActivationFunctionType.Relu
    NCH = BL // FD  # chunks

    with TileContext(nc) as tc:
        with tc.sbuf_pool(name="sb", bufs=1) as sb:
            # ---- warmup first in program order: table preload + PE ramp.
            # junkbf is bf16 zeros: warm matmuls and MLP fillers both use it.
            junkbf = sb.tile([P, P + 256], BF16)
            nc.vector.memset(junkbf[:], 0.0)
            nc.scalar.activation(junkbf[:, 0:8], junkbf[:, 0:8], DERF)
            # pin the m-pool's SBUF region before hT/h2T exist so the first
            # m tiles don't inherit a wait on overlapping dead tiles
            mwarm = sb.tile([P, BL], F16, tag="m", bufs=8, name="mwarm")
            nc.vector.memset(mwarm[:, 0:8], 0.0)
            with tc.psum_pool(name="ps_warm", bufs=1) as ps_warm:
                wt = ps_warm.tile([P, 256], F32, tag="warm")
                for _ in range(8):
                    nc.tensor.matmul(wt[:], junkbf[:, :P], junkbf[:, P:], start=True, stop=True)

            # ---- input DMAs (arrival is gated by DMA-ring startup, not by
            # the issuing sequencer; keep them all on the idle SP queue).
            # xw is split in two in case ring assignment is per-DMA. ----
            xw = sb.tile([3 * IN_DIM, BL], BF16)
            nc.sync.dma_start(out=xw[0:48, :], in_=xw_d[0:48, :])
            nc.sync.dma_start(out=xw[48:96, :], in_=xw_d[48:96, :])
            w1s = sb.tile([3 * IN_DIM, H], BF16)
            nc.sync.dma_start(out=w1s[:], in_=w1s_d[:])
            cst = sb.tile([P, C_END], F32)
            nc.sync.dma_start(out=cst[:], in_=cst_d[:])
            w23 = sb.tile([P, (NJ * NJ + NJ * NOT) * P], F16)
            nc.sync.dma_start(out=w23[:], in_=w23_d[:])
            xbp = sb.tile([P, (BL // P) * IN_DIM], F32)
            nc.sync.dma_start(out=xbp[:], in_=xbp_d[:])
            f16c = sb.tile([P, P + NOT * N_RULES * P], F16)
            nc.sync.dma_start(out=f16c[:], in_=f16c_d[:])

            b1t = cst[:, C_B1:C_B1 + NJ]
            b2t = cst[:, C_B2:C_B2 + NJ]
            scl = cst[:, C_SCL:C_SCL + NOT * N_RULES]
            bia = cst[:, C_BIA:C_BIA + NOT * N_RULES]
            eye16 = f16c[:, :P]
            dgs = f16c[:, P:]

            def W2blk(k, j):
                return w23[:, (k * NJ + j) * P:(k * NJ + j + 1) * P]

            def W3blk(k, j):
                off = NJ * NJ * P
                return w23[:, off + (k * NOT + j) * P:off + (k * NOT + j + 1) * P]

            def relu_bias_chunk(dst, src_psum, bias_col, c, eng):
                cs = slice(c * FD, (c + 1) * FD)
                if eng == 0:
                    nc.vector.tensor_scalar(dst[:, cs], src_psum[:, cs], bias_col, 0.0,
                                            Op.add, Op.max)
                else:
                    nc.scalar.activation(dst[:, cs], src_psum[:, cs], RELU,
                                         bias=bias_col, scale=1.0)

            hT = [None, None]
            h2T = [None, None]
            # ps_a: l2_j0 -> yT0 -> N1 ; ps_b: scratch(fillers) -> yT1p -> D1
            with tc.psum_pool(name="ps_a", bufs=1) as ps_a:
                with tc.psum_pool(name="ps_b", bufs=1) as ps_b:
                    # filler scratch shares ps_b's banks via tag rotation;
                    # nothing reads it so successors allocate instantly
                    wt2 = ps_b.tile([P, FD], F32, tag="bb", name="wt2")

                    def filler(n):
                        for _ in range(n):
                            nc.tensor.matmul(wt2[:, :256], junkbf[:, :P], junkbf[:, P:],
                                             start=True, stop=True)

                    with tc.psum_pool(name="ps_mlp", bufs=2) as ps_mlp:
                        l1s = [None, None]
                        for j in range(NJ):
                            l1 = ps_mlp.tile([P, BL], F32, tag="mlp", name=f"l1_{j}")
                            for c in range(NCH):
                                nc.tensor.matmul(
                                    l1[:, c * FD:(c + 1) * FD],
                                    w1s[:, j * P:(j + 1) * P],
                                    xw[:, c * FD:(c + 1) * FD],
                                    start=True, stop=True,
                                )
                            l1s[j] = l1
                            hT[j] = sb.tile([P, BL], F16, name=f"hT{j}")
                        for c in range(NCH):
                            for j in range(NJ):
                                relu_bias_chunk(hT[j], l1s[j], b1t[:, j:j + 1], c, j % 2)
                        filler(10)  # bridge the relu wait; keep PE clocked up
                        l2s = [None, None]
                        for j in (1, 0):  # j1 first: its relus run on ACT
                            l2 = ps_mlp.tile([P, BL], F32, tag="mlp", name=f"l2_{j}")
                            for c in range(NCH):
                                for k in range(NJ):
                                    nc.tensor.matmul(
                                        l2[:, c * FD:(c + 1) * FD],
                                        W2blk(k, j),
                                        hT[k][:, c * FD:(c + 1) * FD],
                                        start=(k == 0), stop=(k == NJ - 1),
                                    )
                            l2s[j] = l2
                            h2T[j] = sb.tile([P, BL], F16, name=f"h2T{j}")
                        for c in range(NCH):
                            for j in (1, 0):
                                relu_bias_chunk(h2T[j], l2s[j], b2t[:, j:j + 1], c, j % 2)

                        # P[b] = prod_f x[b,f] -> replicated [P, BL]
                        P_rep = sb.tile([P, BL], F32)
                        P_all = sb.tile([P, BL // P], F32)
                        nc.vector.tensor_reduce(
                            P_all[:],
                            xbp.rearrange("p (t f) -> p t f", f=IN_DIM),
                            mybir.AxisListType.X, Op.mult,
                        )
                        P_row = sb.tile([1, BL], F32)
                        for t in range(BL // P):
                            nc.sync.dma_start(out=P_row[0:1, t * P:(t + 1) * P], in_=P_all[:, t:t + 1])
                        nc.gpsimd.partition_broadcast(P_rep[:], P_row[0:1, :])

                    # bridge the relu2 wait before L3.  rhs = h2T[1] makes
                    # these depend on L2-j1's relu so the scheduler cannot
                    # hoist them back before L2 (standalone fillers get
                    # reordered to the earliest idle slot).
                    for _ in range(4):
                        nc.tensor.matmul(wt2[:, :256], junkbf[:, :P], h2T[1][:, :256],
                                         start=True, stop=True)
                    # ---- L3: yT1p FIRST so every PE op the stream's coarse
                    # semaphore waits on is done before rule 0's DERF; yT0
                    # (the first DERF's input) comes last
                    yT1p = ps_b.tile([P, BL], F32, tag="bb", name="yT1p")
                    for c in range(NCH):
                        for k in range(NJ):
                            nc.tensor.matmul(
                                yT1p[:, c * FD:(c + 1) * FD], W3blk(k, 1),
                                h2T[k][:, c * FD:(c + 1) * FD],
                                start=(k == 0), stop=(k == NJ - 1),
                            )
                    yT0 = ps_a.tile([P, BL], F32, tag="aa", name="yT0")
                    for c in range(NCH):
                        for k in range(NJ):
                            nc.tensor.matmul(
                                yT0[:, c * FD:(c + 1) * FD], W3blk(k, 0),
                                h2T[k][:, c * FD:(c + 1) * FD],
                                start=(k == 0), stop=(k == NJ - 1),
                            )
                    # y PSUM->SBUF copies are emitted inside the rule loop,
                    # after rule 0's DERF, so the scheduler cannot gate the
                    # first DERF on them
                    y0s = sb.tile([P, BL], F32, name="y0s")
                    y1s = sb.tile([P, BL], F32, name="y1s")

                    # ---- memberships + D/N + w per o-tile ----
                    with tc.psum_pool(name="ps_c", bufs=1) as ps_c:
                        for ot in range(NOT):
                            if ot == 0:
                                D = ps_c.tile([P, BL], F32, tag="D", name="D0")
                                N = ps_c.tile([P, BL], F32, tag="N", name="N0")
                            else:
                                D = ps_b.tile([P, BL], F32, tag="bb", name="D1")
                                N = ps_a.tile([P, BL], F32, tag="aa", name="N1")
                            for r in range(N_RULES):
                                idx = ot * N_RULES + r
                                # rule 0 of ot0 reads PSUM y (its SBUF copy is
                                # still in flight); everything else reads SBUF
                                ysrc = yT0 if (ot == 0 and r == 0) else (y0s if ot == 0 else y1s)
                                m = sb.tile([P, BL], F16, tag="m", bufs=8, name=f"m{idx}")
                                if ot == NOT - 1 and r == N_RULES - 1:
                                    for c in range(NCH):
                                        cs = slice(c * FD, (c + 1) * FD)
                                        nc.scalar.activation(
                                            m[:, cs], ysrc[:, cs], DERF,
                                            bias=bia[:, idx:idx + 1], scale=scl[:, idx:idx + 1],
                                        )
                                else:
                                    nc.scalar.activation(
                                        m[:], ysrc[:], DERF,
                                        bias=bia[:, idx:idx + 1], scale=scl[:, idx:idx + 1],
                                    )
                                for c in range(NCH):
                                    cs = slice(c * FD, (c + 1) * FD)
                                    nc.tensor.matmul(D[:, cs], eye16, m[:, cs],
                                                     start=(r == 0), stop=(r == N_RULES - 1))
                                    nc.tensor.matmul(N[:, cs], dgs[:, idx * P:(idx + 1) * P], m[:, cs],
                                                     start=(r == 0), stop=(r == N_RULES - 1))
                                if ot == 0 and r == 0:
                                    nc.vector.tensor_copy(y0s[:, 0:FD], yT0[:, 0:FD])
                                    nc.vector.tensor_copy(y0s[:, FD:], yT0[:, FD:])
                                    nc.vector.tensor_copy(y1s[:], yT1p[:])
                            rD = sb.tile([P, BL], F32, tag="rD", bufs=2, name=f"rD{ot}")
                            w = sb.tile([P, BL], F32, tag="w", bufs=2, name=f"w{ot}")
                            o = sb.tile([P, BL], F32, tag="osb", bufs=2, name=f"osb{ot}")
                            if ot < NOT - 1:
                                # fully hidden under the next o-tile's DERFs
                                nc.vector.reciprocal_approx_fast(rD[:], D[:])
                                nc.vector.tensor_tensor(w[:], N[:], rD[:], Op.mult)
                                for _ in range(5):
                                    nc.vector.tensor_tensor(w[:], w[:], w[:], Op.mult)
                                nc.vector.tensor_tensor(o[:], w[:], P_rep[:], Op.mult)
                                nc.sync.dma_start(out=out_d[ot * P:(ot + 1) * P, :], in_=o[:])
                            else:
                                c0 = slice(0, FD)
                                c1 = slice(FD, BL)
                                # DVE: recips, w-mults, ladder c1, o-mults;
                                # ACT: ladder c0 right after its final DERF
                                nc.vector.reciprocal_approx_fast(rD[:, c0], D[:, c0])
                                nc.vector.tensor_tensor(w[:, c0], N[:, c0], rD[:, c0], Op.mult)
                                nc.vector.reciprocal_approx_fast(rD[:, c1], D[:, c1])
                                nc.vector.tensor_tensor(w[:, c1], N[:, c1], rD[:, c1], Op.mult)
                                # balance the ladders: ACT does all 5 of c0
                                # then the last 2 of c1; DVE does c1's first 3
                                # then both output multiplies
                                for _ in range(5):
                                    nc.scalar.activation(w[:, c0], w[:, c0], SQ)
                                for _ in range(3):
                                    nc.vector.tensor_tensor(w[:, c1], w[:, c1], w[:, c1], Op.mult)
                                for _ in range(2):
                                    nc.scalar.activation(w[:, c1], w[:, c1], SQ)
                                nc.vector.tensor_tensor(o[:, c0], w[:, c0], P_rep[:, c0], Op.mult)
                                nc.sync.dma_start(out=out_d[ot * P:(ot + 1) * P, c0], in_=o[:, c0])
                                nc.vector.tensor_tensor(o[:, c1], w[:, c1], P_rep[:, c1], Op.mult)
                                nc.sync.dma_start(out=out_d[ot * P:(ot + 1) * P, c1], in_=o[:, c1])

    nc.finalize()
    _nc_cache = nc
    return nc


def _prepare_in_maps(x, W1, b1, W2, b2, W3, b3, centers, widths, params):
    x = np.ascontiguousarray(x, dtype=np.float32)
    W1 = np.asarray(W1, np.float32); b1 = np.asarray(b1, np.float32)
    W2 = np.asarray(W2, np.float32); b2 = np.asarray(b2, np.float32)
    W3 = np.asarray(W3, np.float32); b3 = np.asarray(b3, np.float32)
    centers = np.asarray(centers, np.float32)
    widths = np.asarray(widths, np.float32)
    params = np.asarray(params, np.float32)

    def pack_w(W, nj_out):
        blocks = []
        for k in range(W.shape[0] // P):
            for j in range(nj_out):
                blocks.append(W[k * P:(k + 1) * P, j * P:(j + 1) * P])
        return np.concatenate(blocks, axis=1)

    w23 = np.ascontiguousarray(
        np.concatenate([pack_w(W2, NJ), pack_w(W3, NOT)], axis=1).astype(np.float16))

    b1t = b1.reshape(NJ, P).T
    b2t = b2.reshape(NJ, P).T
    inv = (1.0 / widths).astype(np.float32)
    biasf = ((b3[:, None] - centers) * inv).astype(np.float32)
    scl = inv.reshape(NOT, P, N_RULES).transpose(1, 0, 2).reshape(P, NOT * N_RULES)
    bia = biasf.reshape(NOT, P, N_RULES).transpose(1, 0, 2).reshape(P, NOT * N_RULES)
    cst = np.ascontiguousarray(np.concatenate([b1t, b2t, scl, bia], axis=1))

    ph = params.astype(np.float16)
    dgs = np.zeros((P, NOT * N_RULES * P), np.float16)
    for ot in range(NOT):
        for r in range(N_RULES):
            idx = ot * N_RULES + r
            dgs[:, idx * P:(idx + 1) * P] = np.diag(ph[ot * P:(ot + 1) * P, r])
    f16c = np.ascontiguousarray(np.concatenate([np.eye(P, dtype=np.float16), dgs], axis=1))

    # L1 bf16 hi/lo stacking: y1 = W1h.T@xh + W1l.T@xh + W1h.T@xl
    W1h = W1.astype(ml_bf16)
    W1l = (W1 - W1h.astype(np.float32)).astype(ml_bf16)
    w1s = np.ascontiguousarray(np.concatenate([W1h, W1l, W1h], axis=0))  # [96, H]

    in_maps = []
    for i in range(N_CORES):
        xs = x[i * BL:(i + 1) * BL]                              # [BL, 32]
        xT = np.ascontiguousarray(xs.T)                          # [32, BL]
        xh = xT.astype(ml_bf16)
        xl = (xT - xh.astype(np.float32)).astype(ml_bf16)
        xw = np.ascontiguousarray(np.concatenate([xh, xh, xl], axis=0))  # [96, BL]
        xbp = np.ascontiguousarray(
            xs.reshape(BL // P, P, IN_DIM).transpose(1, 0, 2).reshape(P, -1))
        in_maps.append(dict(xw=xw, w1s=w1s, cst=cst, w23=w23, f16c=f16c, xbp=xbp))
    return in_maps


def run(trace=False, **inputs):
    nc = _build()
    in_maps = _prepare_in_maps(**inputs)
    res = run_bass_kernel_spmd(nc, in_maps, core_ids=list(range(N_CORES)), trace=trace)
    outs = [res.results[i]["out"].T for i in range(N_CORES)]     # each [BL, O]
    full = np.ascontiguousarray(np.concatenate(outs, axis=0), dtype=np.float32)
    return full, res


def kernel(**inputs) -> np.ndarray:
    full, _ = run(trace=False, **inputs)
    return full
